# revision 18
# baseline (speedup 1.0000x reference)
# Trainium2 Bass kernel for nn_ChannelTail (channel self-attention tail).
#
# Math (per batch element b):
#   value = w_value @ x_b + b_value            [256, HW]
#   A     = softmax(energy_b, axis=-1)         [256, 256]
#   out   = w_re @ (A @ value) + b_re          [512, HW]
#   y     = gamma * out + 2 * x_b
#
# Key algebraic fusion: w_re @ (A @ value) == (w_re @ A) @ value, and
# W2 = w_re @ A is only [512, 256] per batch. So instead of three big
# GEMMs over HW=16384 pixels we do two:
#   value' = w_value @ x_b + b_value            (K=512 -> M=256)
#   y      = (gamma*W2) @ value' + gamma*b_re + 2*x
# with W2 computed once per core from softmax(energy).
#
# Sharding: data-parallel over batch. 8 batch elements, 8 cores, one
# batch element per core. Weights replicated. No collectives.
#
# HW-microbenchmarked choices:
#  - fp32r matmuls (full PE speed at N=512, no operand casts needed)
#  - PSUM drained only by ScalarE Identity+bias (~614ns/[128,512];
#    DVE PSUM reads are ~3x slower)
#  - DVE does the SBUF-only epilogue stt out=(2x+t) (~403ns)
#  - GPSIMD never does elementwise (8-17x slower than the cost model)
#  - DMA in 4MiB super-tiles (8KB contiguous runs per descriptor);
#    loads on the SP HWDGE ring, stores on the ACT ring

import numpy as np
from contextlib import ExitStack

B, C_IN, C_INT, H, W = 8, 512, 256, 128, 128
HW = H * W            # 16384
NT = 512              # pixels per compute sub-tile (one PSUM bank fp32)
NCORES = 8
P = 128               # partitions
KI = C_IN // P        # 4 input-channel chunks
KM = C_INT // P       # 2 intermediate-channel chunks

_built = None
GEMM_DTYPE = "f32r"   # "f32r" (fastest, ~1.8e-4 rel err) or "bf16" (~1.1e-5)


def _build(reps=1):
    """Trace + schedule + compile the Bass program. Returns nc.

    reps>1 repeats the main pixel loop (same data) for benchmarking:
    steady-state time per rep = (t(R2)-t(R1))/(R2-R1).
    """
    import concourse.bass as bass
    import concourse.mybir as mybir
    import concourse.tile as tile
    from concourse import bacc
    from concourse.bass import ds

    fp32 = mybir.dt.float32
    f32r = mybir.dt.float32r
    bf16 = mybir.dt.bfloat16
    gdt = bf16 if GEMM_DTYPE == "bf16" else f32r
    xdt = fp32 if GEMM_DTYPE == "bf16" else f32r
    # x-load super-tile: as large as SBUF allows -> longer contiguous runs
    # (16KB at 4096 px) -> higher load bandwidth. Stores go out per 512-px
    # sub-tile (1MiB) -- store BW is insensitive to run size (measured).
    SUP = 2048 if GEMM_DTYPE == "bf16" else 4096
    N_SUP = HW // SUP
    SUBS = SUP // NT
    AF = mybir.ActivationFunctionType
    OP = mybir.AluOpType
    AX = mybir.AxisListType

    nc = bacc.Bacc("TRN2", target_bir_lowering=False, debug=False,
                   num_devices=NCORES)

    energy = nc.dram_tensor("energy", [C_INT, C_INT], fp32, kind="ExternalInput").ap()
    x_d = nc.dram_tensor("x", [C_IN, HW], xdt, kind="ExternalInput").ap()
    wvT_d = nc.dram_tensor("w_valueT", [C_IN, C_INT], gdt, kind="ExternalInput").ap()
    wrT_d = nc.dram_tensor("w_reT", [C_INT, C_IN], gdt, kind="ExternalInput").ap()
    bval_d = nc.dram_tensor("b_value_t", [P, KM], fp32, kind="ExternalInput").ap()
    bre_d = nc.dram_tensor("b_re_t", [P, KI], fp32, kind="ExternalInput").ap()
    gam_d = nc.dram_tensor("gamma", [1, 1], fp32, kind="ExternalInput").ap()
    out_d = nc.dram_tensor("out", [C_IN, HW], fp32, kind="ExternalOutput").ap()

    # chunked DRAM views: row (q*128 + p) -> [p, q, cols]
    xv = x_d.rearrange("(q p) n -> p q n", p=P)     # [128, 4, HW]
    ov = out_d.rearrange("(q p) n -> p q n", p=P)   # [128, 4, HW]

    with tile.TileContext(nc) as tc, ExitStack() as ctx:
        const = ctx.enter_context(tc.tile_pool(name="const", bufs=1))

        # ---------- load constants (SWDGE; setup only) ----------
        e_sb = []
        for i in range(KM):
            t = const.tile([P, C_INT], fp32, tag=f"e{i}", name=f"e{i}")
            nc.gpsimd.dma_start(t[:], energy.rearrange("(k p) m -> k p m", p=P)[i])
            e_sb.append(t)
        wvT_sb = []
        for k in range(KI):
            t = const.tile([P, C_INT], gdt, tag=f"wvT{k}", name=f"wvT{k}")
            nc.gpsimd.dma_start(t[:], wvT_d.rearrange("(k p) m -> k p m", p=P)[k])
            wvT_sb.append(t)
        wrT_sb = []
        for k in range(KM):
            t = const.tile([P, C_IN], gdt, tag=f"wrT{k}", name=f"wrT{k}")
            nc.gpsimd.dma_start(t[:], wrT_d.rearrange("(k p) m -> k p m", p=P)[k])
            wrT_sb.append(t)
        bval_sb = const.tile([P, KM], fp32, tag="bval")
        nc.gpsimd.dma_start(bval_sb[:], bval_d)
        bre_sb = const.tile([P, KI], fp32, tag="bre")
        nc.gpsimd.dma_start(bre_sb[:], bre_d)
        g_bc = const.tile([P, 1], fp32, tag="gbc")
        nc.gpsimd.dma_start(g_bc[:], gam_d.to_broadcast([P, 1]))

        # bias2 = gamma * b_re   [128, 4]
        bias2 = const.tile([P, KI], fp32, tag="bias2")
        nc.vector.tensor_scalar_mul(bias2[:], bre_sb[:], g_bc[:])

        # ---------- softmax(energy) -> A (fp32) ----------
        A_sb = []
        for i in range(KM):
            negmax = const.tile([P, 1], fp32, tag=f"negmax{i}", name=f"negmax{i}")
            nc.vector.tensor_reduce(negmax[:], e_sb[i][:], axis=AX.X, op=OP.max,
                                    negate=True)
            pexp = const.tile([P, C_INT], fp32, tag=f"pexp{i}", name=f"pexp{i}")
            sums = const.tile([P, 1], fp32, tag=f"sums{i}", name=f"sums{i}")
            nc.scalar.activation(pexp[:], e_sb[i][:], AF.Exp, bias=negmax[:],
                                 scale=1.0, accum_out=sums[:])
            rec = const.tile([P, 1], fp32, tag=f"rec{i}", name=f"rec{i}")
            nc.vector.reciprocal(rec[:], sums[:])
            a = const.tile([P, C_INT], gdt, tag=f"A{i}", name=f"A{i}")
            nc.vector.tensor_scalar_mul(a[:], pexp[:], rec[:])
            A_sb.append(a)

        # ---------- W2T = A @ w_reT, scaled by gamma ----------
        # W2 = w_re @ A  =>  W2T[i, o] = sum_j A[j, i] * w_reT[j, o]
        # lhsT = A chunk [k=j, m=i], rhs = w_reT chunk [k=j, n=o]
        W2T_sb = []
        with tc.tile_pool(name="psum_setup", bufs=2, space="PSUM") as psum_setup:
            for m in range(KM):
                ps = psum_setup.tile([P, C_IN], fp32, tag="w2t_ps")
                for k in range(KM):
                    nc.tensor.matmul(ps[:],
                                     A_sb[k][:, m * P:(m + 1) * P],
                                     wrT_sb[k][:],
                                     start=(k == 0), stop=(k == KM - 1))
                w2t = const.tile([P, C_IN], gdt, tag=f"W2T{m}", name=f"W2T{m}")
                nc.vector.tensor_scalar_mul(w2t[:], ps[:], g_bc[:])
                W2T_sb.append(w2t)

        # ---------- main loop over pixel super-tiles ----------
        px = ctx.enter_context(tc.tile_pool(
            name="px", bufs=3 if GEMM_DTYPE == "bf16" else 2))
        if GEMM_DTYPE == "bf16":
            pxb = ctx.enter_context(tc.tile_pool(name="pxb", bufs=1))
        pval = ctx.enter_context(tc.tile_pool(name="pval", bufs=4))
        pt = ctx.enter_context(tc.tile_pool(name="pt", bufs=6))
        pout = ctx.enter_context(tc.tile_pool(name="pout", bufs=3))
        ps_val = ctx.enter_context(tc.tile_pool(name="ps_val", bufs=4, space="PSUM"))
        ps_out = ctx.enter_context(tc.tile_pool(name="ps_out", bufs=4, space="PSUM"))

        for s in range(N_SUP * reps):
            s = s % N_SUP
            x_t = px.tile([P, KI, SUP], xdt, tag="x")
            nc.sync.dma_start(x_t[:], xv[:, :, ds(s * SUP, SUP)])
            if GEMM_DTYPE == "bf16":
                xb = pxb.tile([P, KI, SUP], bf16, tag="xb")
                nc.vector.tensor_copy(
                    xb.rearrange("p q n -> p (q n)")[:],
                    x_t.rearrange("p q n -> p (q n)")[:])
                g1rhs = xb
            else:
                g1rhs = x_t

            for u in range(SUBS):
                lo = u * NT
                # GEMM1: value' = w_value @ x + b_value
                val = []
                for m in range(KM):
                    pv = ps_val.tile([P, NT], fp32, tag="pv")
                    for k in range(KI):
                        nc.tensor.matmul(
                            pv[:],
                            wvT_sb[k][:, m * P:(m + 1) * P],
                            g1rhs[:, k, lo:lo + NT],
                            start=(k == 0), stop=(k == KI - 1))
                    v = pval.tile([P, NT], gdt, tag="val")
                    nc.scalar.activation(v[:], pv[:], AF.Identity,
                                         bias=bval_sb[:, m:m + 1], scale=1.0)
                    val.append(v)

                # GEMM2 + epilogue: out = (gamma*W2 @ value' + bias2) + 2x
                out_u = pout.tile([P, KI, NT], fp32, tag="out")
                for mo in range(KI):
                    po = ps_out.tile([P, NT], fp32, tag="po")
                    for k in range(KM):
                        nc.tensor.matmul(
                            po[:],
                            W2T_sb[k][:, mo * P:(mo + 1) * P],
                            val[k][:],
                            start=(k == 0), stop=(k == KM - 1))
                    t = pt.tile([P, NT], fp32, tag="t")
                    nc.scalar.activation(t[:], po[:], AF.Identity,
                                         bias=bias2[:, mo:mo + 1], scale=1.0)
                    nc.vector.scalar_tensor_tensor(
                        out_u[:, mo, :],
                        x_t[:, mo, lo:lo + NT].bitcast(fp32)
                        if GEMM_DTYPE == "f32r" else x_t[:, mo, lo:lo + NT],
                        2.0, t[:], op0=OP.mult, op1=OP.add)

                # per-sub store (1MiB) on the ACT HWDGE ring; store BW is
                # insensitive to run length so fine granularity is free and
                # keeps the SBUF staging small
                nc.scalar.dma_start(ov[:, :, ds(s * SUP + lo, NT)], out_u[:])

    nc.compile()
    return nc


def _get_built(reps=1):
    global _built
    if _built is None:
        _built = {}
    if reps not in _built:
        _built[reps] = _build(reps)
    return _built[reps]


def _prep_in_maps(energy, x, w_value, b_value, w_re, b_re, gamma):
    wvT = np.ascontiguousarray(np.asarray(w_value, np.float32).T)
    wrT = np.ascontiguousarray(np.asarray(w_re, np.float32).T)
    if GEMM_DTYPE == "bf16":
        import ml_dtypes
        wvT = wvT.astype(ml_dtypes.bfloat16)
        wrT = wrT.astype(ml_dtypes.bfloat16)
    bval_t = np.ascontiguousarray(
        np.asarray(b_value, np.float32).reshape(KM, P).T)
    bre_t = np.ascontiguousarray(np.asarray(b_re, np.float32).reshape(KI, P).T)
    gam = np.asarray(gamma, np.float32).reshape(1, 1)
    x = np.asarray(x, np.float32)
    energy = np.asarray(energy, np.float32)

    in_maps = []
    for b in range(NCORES):
        in_maps.append({
            "energy": np.ascontiguousarray(energy[b]),
            "x": np.ascontiguousarray(x[b].reshape(C_IN, HW)),
            "w_valueT": wvT,
            "w_reT": wrT,
            "b_value_t": bval_t,
            "b_re_t": bre_t,
            "gamma": gam,
        })
    return in_maps


def run(inputs, trace=False, **kw):
    """Run on 8 cores; returns (output [B,C_IN,H,W], BassKernelResults)."""
    from concourse.bass_utils import run_bass_kernel_spmd
    nc = _get_built()
    in_maps = _prep_in_maps(**inputs)
    res = run_bass_kernel_spmd(nc, in_maps, core_ids=list(range(NCORES)),
                               trace=trace, **kw)
    out = np.stack([r["out"] for r in res.results])
    return out.reshape(B, C_IN, H, W).astype(np.float32), res


def kernel(**inputs) -> np.ndarray:
    out, _ = run(inputs, trace=False)
    return out
